# revision 32
# baseline (speedup 1.0000x reference)
"""Trainium2 Bass kernel for nn_MultiHeadSelfAttention_62646392979761.

Math (per the buggy-einsum reference): per position s, heads attend to heads:
  Q,K,V = x@W{q,k,v}.T + b  (N,S,H,D);  scores[s] = Q[s]K[s]^T/8 (16x16);
  A = softmax_j;  AV[s] = A[s]V[s];  out2 = scrambled flat reshape;
  final = out2@Wo.T + bo.

Sharding: 8 cores x 2048 rows of the flattened (16384, 1024) x. Attention is
position-local; the scramble groups 16 consecutive positions, which never
cross a 2048-row shard. Zero cross-core communication.

Per-core pipeline (positions in 4 groups of 512, each 4 subtiles of 128).
Indices: position s = 128*sub + 16*G + w, pair-half p = w%2, c' = w//2%8,
pair g = 8*G + c' (u = g%4, gg = g//4), head i = 2c+a.
  1. QT/KT projections transposed (stationary = weight chunk, moving = xT):
     psum [128 f=(a,d), 512 s] per f-chunk; DVE bias evac -> qt/kt [(a,d),(c,s)].
  2. V projection natural (stationary = xT chunk): vn [128 s, 1024 (j,d)].
  3. QSTK [(p,d), 16g+i] / KBLK [(p,d), 32g+16p+j block-diag] via 4 strided
     DVE copies each from qt/kt. VBLK [(p,j), 128g+64p+d block-diag] via 2
     SBUF->SBUF gather DMAs from vn. Structural zeros memset on first use.
  4. Scores^T: 64 pair matmuls (k=128, m=32 col-rotated, n=16, stationary
     KBLK slab, moving QSTK slice) -> sco psum [(u,p,j), (gg,i)].
  5. Softmax: ACT exp -> E bf16; Z = mask^T-matmul (sums j over partitions);
     DVE reciprocal; Zb = maskT-matmul broadcast; A = E*Zb (DVE).
  6. AV: 64 pair matmuls (k=32 row-rotated, m=128, n=16, stationary VBLK
     slab, moving A slice) -> avp psum [(p,d), 16g+i].
  7. ACT copies avp -> out2T chunks [128 (a,d), 2048 (16G+i)] (c' = chunk).
  8. Final projection: stationary WoPT chunks, moving out2T -> psum
     [128 f, 512 s'], + bo (DVE) -> out_d (1024, 2048) f32.
Host: transposes x/weights, permutes Wo rows, post-scatters out columns
(col = 16G + i -> row i*256 + s0/16 + G).
"""

import math
import numpy as np
import ml_dtypes

ROWS = 2048
NB, SB, EB, HB, DB = 4, 4096, 1024, 16, 64

_CACHE = {}


def _split_waits_json(bir_bytes):
    """This env's walrus accepts only ONE embedded sync-wait per TPB
    instruction (NEURON_ISA_TPB_EVENTS has a single wait slot) but Tile emits
    several. Split excess on_wait entries onto standalone EventSemaphore
    instructions inserted just before, on the same engine — semantically
    identical on in-order engine queues."""
    import json
    d = json.loads(bir_bytes)
    for fn in d.get('functions', []):
        for bb in (fn.get('basic_blocks') or fn.get('blocks') or []):
            out = []
            for inst in bb.get('instructions', []):
                si = inst.get('sync_info')
                w = (si or {}).get('on_wait') or []
                if len(w) > 1:
                    for k, extra in enumerate(w[:-1]):
                        out.append({
                            'debug': inst.get('debug', 0),
                            'engine': inst['engine'],
                            'ins': [], 'outs': [],
                            'name': f"{inst['name']}-sw{k}",
                            'opcode': 'EventSemaphore',
                            'sync_info': {'on_wait': [extra], 'on_update': []},
                        })
                    si['on_wait'] = [w[-1]]
                out.append(inst)
            bb['instructions'] = out
    return json.dumps(d).encode()


def _install_birpatch():
    import concourse.bass_utils as bu
    import concourse.bass2jax as b2j
    if getattr(bu.compile_bir_kernel, '_waitsplit', False):
        return
    orig = bu.compile_bir_kernel

    def patched(bir_json, tmpdir, neff_name="file.neff"):
        return orig(_split_waits_json(bir_json), tmpdir, neff_name)

    patched._waitsplit = True
    bu.compile_bir_kernel = patched
    b2j.compile_bir_kernel = patched


def _build_bass(reps=1):
    import os
    import concourse.bass as bass
    import concourse.tile as tile
    from concourse import mybir
    _ab = set(os.environ.get("KABLATE", "").split(","))

    bf16 = mybir.dt.bfloat16
    f32 = mybir.dt.float32
    AF = mybir.ActivationFunctionType

    nc = bass.Bass(trn_type="TRN2")
    xt_d = nc.declare_dram_parameter("xt", [1024, ROWS], bf16, isOutput=False)
    wqT_d = nc.declare_dram_parameter("wqt", [1024, 1024], bf16, isOutput=False)
    wkT_d = nc.declare_dram_parameter("wkt", [1024, 1024], bf16, isOutput=False)
    wvT_d = nc.declare_dram_parameter("wvt", [1024, 1024], bf16, isOutput=False)
    woT_d = nc.declare_dram_parameter("wot", [1024, 1024], bf16, isOutput=False)
    bias_d = nc.declare_dram_parameter("bias", [128, 1048], f32, isOutput=False)
    mask_d = nc.declare_dram_parameter("mask", [128, 32], bf16, isOutput=False)
    maskT_d = nc.declare_dram_parameter("maskt", [32, 128], bf16, isOutput=False)
    out_d = nc.declare_dram_parameter("out", [1024, ROWS], f32, isOutput=True)

    from contextlib import ExitStack
    with ExitStack() as ctx:
        tc = ctx.enter_context(tile.TileContext(nc))
        const = ctx.enter_context(tc.tile_pool(name="const", bufs=1))
        work = ctx.enter_context(tc.tile_pool(name="work", bufs=2))
        vnp = ctx.enter_context(tc.tile_pool(name="vnp", bufs=2))
        ppj = ctx.enter_context(tc.tile_pool(name="ppj", bufs=2, space="PSUM"))
        pvj = ctx.enter_context(tc.tile_pool(name="pvj", bufs=1, space="PSUM"))
        psc = ctx.enter_context(tc.tile_pool(name="psc", bufs=2, space="PSUM"))
        ps1 = ctx.enter_context(tc.tile_pool(name="ps1", bufs=1, space="PSUM"))
        drp = ctx.enter_context(tc.tile_pool(name="drp", bufs=4, space="DRAM"))

        if reps != 1:
            ctx.enter_context(tc.For_i(0, reps))

        # ---- persistent tensors (reloaded every rep for honest timing) ----
        wq_sb = const.tile([128, 8192], bf16, tag="wq")
        wk_sb = const.tile([128, 8192], bf16, tag="wk")
        wv_sb = const.tile([128, 8192], bf16, tag="wv")
        wo_sb = const.tile([128, 8192], bf16, tag="wo")
        bias_sb = const.tile([128, 1048], f32, tag="bias")
        mask_sb = const.tile([128, 32], bf16, tag="mask")
        maskT_sb = const.tile([32, 128], bf16, tag="maskt")
        out2t = const.tile([128, 16384], bf16, tag="o2t", name="o2t")
        kblks = [const.tile([128, 2048], bf16, tag=f"kblk{b}", name=f"kblk{b}")
                 for b in range(2)]
        vblks = [const.tile([32, 8192], bf16, tag=f"vblk{b}", name=f"vblk{b}")
                 for b in range(2)]
        for b in range(2):
            nc.gpsimd.memset(kblks[b][:], 0.0)
            nc.gpsimd.memset(vblks[b][:], 0.0)

        # weight layout: sb[p, 1024*ce + 128*cf + m] = W_T[128*ce + p, 128*cf + m]
        for sb, d in ((wq_sb, wqT_d), (wk_sb, wkT_d), (wv_sb, wvT_d), (wo_sb, woT_d)):
            nc.sync.dma_start(
                sb[:].rearrange("p (ce f) -> p ce f", ce=8),
                d[:].rearrange("(ce p) f -> p ce f", ce=8),
            )
        nc.sync.dma_start(bias_sb[:], bias_d[:])
        nc.sync.dma_start(mask_sb[:], mask_d[:])
        nc.sync.dma_start(maskT_sb[:], maskT_d[:])
        bqT = bias_sb[:, 0:8]
        bkT = bias_sb[:, 8:16]
        boT = bias_sb[:, 16:24]
        bvR = bias_sb[:, 24:1048]

        def _emit_final(fg):
            for cf in range(8):
                pf = ps1.tile([128, 512], f32, tag="zpf", name="pf")
                for cp in range(8):
                    nc.tensor.matmul(
                        pf[:], wo_sb[:, 1024 * cp + 128 * cf:1024 * cp + 128 * (cf + 1)],
                        out2t[:, 2048 * cp + 512 * fg:2048 * cp + 512 * (fg + 1)],
                        start=(cp == 0), stop=(cp == 7))
                osb = work.tile([128, 512], f32, tag="osb")
                nc.vector.tensor_scalar_add(osb[:], pf[:], boT[:, cf:cf + 1])
                nc.sync.dma_start(out_d[128 * cf:128 * (cf + 1), 512 * fg:512 * (fg + 1)], osb[:])

        for sg in range(4):
            # ---- load xT group: [128, (ce, s 512)] ----
            xt_sb = work.tile([128, 4096], bf16, tag="xt")
            nc.sync.dma_start(
                xt_sb[:].rearrange("p (ce s) -> p ce s", ce=8),
                xt_d[:].rearrange("(ce p) s -> p ce s", ce=8)[:, :, 512 * sg:512 * (sg + 1)],
            )
            # ---- QT / KT projections (transposed: stationary = weights) ----
            qt_sg = work.tile([128, 4096], bf16, tag="qt")
            kt_sg = work.tile([128, 4096], bf16, tag="kt")
            for w_sb, bT, dst in ((wq_sb, bqT, qt_sg), (wk_sb, bkT, kt_sg)):
                for cf in range(8):
                    pp = ppj.tile([128, 512], f32, tag="pp", name="pp")
                    for ce in range(8):
                        nc.tensor.matmul(
                            pp[:], w_sb[:, 1024 * ce + 128 * cf:1024 * ce + 128 * (cf + 1)],
                            xt_sb[:, 512 * ce:512 * (ce + 1)],
                            start=(ce == 0), stop=(ce == 7))
                    nc.vector.tensor_scalar_add(
                        dst[:, 512 * cf:512 * (cf + 1)], pp[:], bT[:, cf:cf + 1])
            # ---- V projection (natural: stationary = xT chunk) ----
            vns = []
            for sl in range(4):
                vn = vnp.tile([128, 1024], bf16, tag="vn")
                for h in range(2):
                    pv = pvj.tile([128, 512], f32, tag="pv", name="pv")
                    for ce in range(8):
                        nc.tensor.matmul(
                            pv[:],
                            xt_sb[:, 512 * ce + 128 * sl:512 * ce + 128 * (sl + 1)],
                            wv_sb[:, 1024 * ce + 512 * h:1024 * ce + 512 * (h + 1)],
                            start=(ce == 0), stop=(ce == 7))
                    nc.vector.tensor_add(vn[:, 512 * h:512 * (h + 1)], pv[:],
                                         bvR[:, 512 * h:512 * (h + 1)])
                vdr = drp.tile([128, 1024], bf16, tag="vdr")
                nc.sync.dma_start(vdr[:], vn[:])
                vns.append(vdr)

            for sl in range(4):
                sub = 4 * sg + sl
                vdr = vns[sl]
                # ---- VBLK [32 (p,j), 128g+64p+d] via 2 gather DMAs (DRAM src) ----
                vblk = vblks[sub % 2]
                for p in range(2 if "novblk" not in _ab else 0):
                    nc.sync.dma_start(
                        vblk[:][16 * p:16 * (p + 1), :].rearrange(
                            "P (g two d) -> P g two d", g=64, two=2)[:, :, p, :],
                        vdr[:].rearrange("(G cp t) (j d) -> t j G cp d",
                                         G=8, cp=8, t=2, j=16)[p],
                    )
                # ---- QSTK / KBLK via 4 strided DVE copies each ----
                qstk = work.tile([128, 1024], bf16, tag="qstk", name="qstk")
                kblk = kblks[sub % 2]
                for p in range(2):
                    for a in range(2):
                        src_q = qt_sg[:][64 * a:64 * (a + 1), :].rearrange(
                            "P (c sl G cp t) -> P sl t G cp c",
                            c=8, sl=4, G=8, cp=8, t=2)[:, sl, p]
                        dst_q = qstk[:][64 * p:64 * (p + 1), :].rearrange(
                            "P (G cp c t) -> P t G cp c", G=8, cp=8, c=8, t=2)[:, a]
                        nc.vector.tensor_copy(dst_q, src_q)
                        src_k = kt_sg[:][64 * a:64 * (a + 1), :].rearrange(
                            "P (c sl G cp t) -> P sl t G cp c",
                            c=8, sl=4, G=8, cp=8, t=2)[:, sl, p]
                        dst_k = kblk[:][64 * p:64 * (p + 1), :].rearrange(
                            "P (G cp two c t) -> P two t G cp c",
                            G=8, cp=8, two=2, c=8, t=2)[:, p, a]
                        nc.vector.tensor_copy(dst_k, src_k)
                # ---- scores^T: 64 pair matmuls ----
                sco = psc.tile([128, 256], f32, tag="sco", name="sco")
                if "nosco" in _ab:
                    nc.scalar.activation(sco[:], qstk[:, 0:256], func=AF.Copy)
                else:
                    for g in range(64):
                        u, gg = g % 4, g // 4
                        nc.tensor.matmul(
                            sco[32 * u:32 * (u + 1), 16 * gg:16 * (gg + 1)],
                            kblk[:, 32 * g:32 * (g + 1)],
                            qstk[:, 16 * g:16 * (g + 1)],
                            start=True, stop=True, tile_position=(0, 32 * u))
                # ---- softmax ----
                e_sb = work.tile([128, 256], bf16, tag="esb")
                nc.scalar.activation(e_sb[:], sco[:], func=AF.Exp)
                a_sb = work.tile([128, 256], bf16, tag="asb")
                if "noz" in _ab:
                    nc.vector.tensor_copy(a_sb[:], e_sb[:])
                else:
                    zp = ps1.tile([128, 512], f32, tag="zpf", name="zp")
                    nc.tensor.matmul(zp[0:32, 0:256], mask_sb[:], e_sb[:],
                                     start=True, stop=True)
                    zr = work.tile([32, 256], bf16, tag="zr")
                    nc.vector.memset(zr[:], 0.0)
                    with nc.allow_low_precision(reason="1/Z in bf16: 0.4% on softmax scale"):
                        nc.vector.reciprocal(zr[0:8, :], zp[0:8, 0:256])
                    nc.tensor.matmul(zp[:, 256:512], maskT_sb[:], zr[:],
                                     start=True, stop=True)
                    nc.vector.tensor_mul(a_sb[:], e_sb[:], zp[:, 256:512])
                # ---- A2: bands stacked at partition base 0 ----
                a2 = work.tile([32, 1024], bf16, tag="a2")
                for u in range(4):
                    nc.vector.tensor_copy(a2[:, 256 * u:256 * (u + 1)],
                                          a_sb[32 * u:32 * (u + 1), :])
                # ---- AV: 64 pair matmuls ----
                avp = ps1.tile([128, 1024], f32, tag="avp", name="avp")
                if "noav" in _ab:
                    for q in range(4):
                        nc.scalar.activation(avp[:, 256 * q:256 * (q + 1)], a_sb[:],
                                             func=AF.Copy)
                else:
                    for g in range(64):
                        u, gg = g % 4, g // 4
                        nc.tensor.matmul(
                            avp[:, 16 * g:16 * (g + 1)],
                            vblk[:, 128 * g:128 * (g + 1)],
                            a2[:, 256 * u + 16 * gg:256 * u + 16 * (gg + 1)],
                            start=True, stop=True)
                # ---- evac to out2T (one strided ACT op) ----
                nc.scalar.activation(
                    out2t[:].rearrange("P (cp sb G i) -> P sb cp G i",
                                       cp=8, sb=16, G=8, i=16)[:, sub],
                    avp[:].rearrange("P (G cp i) -> P cp G i", G=8, cp=8),
                    func=AF.Copy)

            # ---- final projection for this group's s' columns ----
            _emit_final(sg)
    return nc


def _host_prep(x, Wq, bq, Wk, bk, Wv, bv, Wo, bo):
    """Returns per-core input maps."""
    xf = np.ascontiguousarray(x.reshape(NB * SB, EB))
    WqT = np.ascontiguousarray((Wq / 8.0).T).astype(ml_dtypes.bfloat16)
    WkT = np.ascontiguousarray(Wk.T).astype(ml_dtypes.bfloat16)
    WvT = np.ascontiguousarray(Wv.T).astype(ml_dtypes.bfloat16)
    WoPT = np.zeros((1024, 1024), np.float32)
    for cp in range(8):
        for a in range(2):
            w = 2 * cp + a
            WoPT[128 * cp + 64 * a:128 * cp + 64 * a + 64, :] = Wo[:, 64 * w:64 * (w + 1)].T
    WoPT = WoPT.astype(ml_dtypes.bfloat16)
    bias = np.zeros((128, 1048), np.float32)
    bias[:, 0:8] = (bq / 8.0).reshape(8, 128).T
    bias[:, 8:16] = bk.reshape(8, 128).T
    bias[:, 16:24] = bo.reshape(8, 128).T
    bias[:, 24:1048] = np.tile(bv[None, :], (128, 1))
    MASK = np.zeros((128, 32), np.float32)
    for u in range(4):
        for p in range(2):
            MASK[32 * u + 16 * p:32 * u + 16 * (p + 1), 2 * u + p] = 1.0
    MASKb = MASK.astype(ml_dtypes.bfloat16)
    MASKTb = np.ascontiguousarray(MASK.T).astype(ml_dtypes.bfloat16)
    in_maps = []
    for core in range(8):
        n, s0 = core // 2, (core % 2) * ROWS
        xs = xf[n * SB + s0:n * SB + s0 + ROWS]
        xT = np.ascontiguousarray(xs.T).astype(ml_dtypes.bfloat16)
        in_maps.append({"xt": xT, "wqt": WqT, "wkt": WkT, "wvt": WvT,
                        "wot": WoPT, "bias": bias, "mask": MASKb, "maskt": MASKTb})
    return in_maps


def _gather_out(core_outs):
    """core_outs: list of 8 per-core 'out' arrays (1024, 2048) -> full (N,S,E).
    out col = 16*G + i -> row i*256 + s0/16 + G."""
    out = np.zeros((NB, SB, EB), np.float32)
    cols = np.arange(ROWS)
    G, i = cols // 16, cols % 16
    for core in range(8):
        n, s0 = core // 2, (core % 2) * ROWS
        fT = np.asarray(core_outs[core])  # (1024, 2048)
        rows = i * 256 + (s0 // 16 + G)
        out[n, rows, :] = fT.T
    return out


def kernel(x, Wq, bq, Wk, bk, Wv, bv, Wo, bo):
    _install_birpatch()
    from concourse.bass_utils import run_bass_kernel_spmd

    if "nc" not in _CACHE:
        _CACHE["nc"] = _build_bass()
    nc = _CACHE["nc"]
    in_maps = _host_prep(np.asarray(x, np.float32), *[np.asarray(a, np.float32)
                         for a in (Wq, bq, Wk, bk, Wv, bv, Wo, bo)])
    res = run_bass_kernel_spmd(nc, in_maps, list(range(8)))
    return _gather_out([res.results[core]["out"] for core in range(8)])


# revision 33
# speedup vs baseline: 1.0271x; 1.0271x over previous
"""Trainium2 Bass kernel for nn_MultiHeadSelfAttention_62646392979761.

Math (per the buggy-einsum reference): per position s, heads attend to heads:
  Q,K,V = x@W{q,k,v}.T + b  (N,S,H,D);  scores[s] = Q[s]K[s]^T/8 (16x16);
  A = softmax_j;  AV[s] = A[s]V[s];  out2 = scrambled flat reshape;
  final = out2@Wo.T + bo.

Sharding: 8 cores x 2048 rows of the flattened (16384, 1024) x. Attention is
position-local; the scramble groups 16 consecutive positions, which never
cross a 2048-row shard. Zero cross-core communication.

Per-core pipeline (positions in 4 groups of 512, each 4 subtiles of 128).
Indices: position s = 128*sub + 16*G + w, pair-half p = w%2, c' = w//2%8,
pair g = 8*G + c' (u = g%4, gg = g//4), head i = 2c+a.
  1. QT/KT projections transposed (stationary = weight chunk, moving = xT):
     psum [128 f=(a,d), 512 s] per f-chunk; DVE bias evac -> qt/kt [(a,d),(c,s)].
  2. V projection natural (stationary = xT chunk): vn [128 s, 1024 (j,d)],
     bounced to DRAM for the VBLK gather.
  3. QSTK [(p,d), 16g+i] / KBLK [(p,d), 32g+16p+j block-diag] via 4 strided
     DVE copies each from qt/kt. VBLK [32 (p,j), 128g+64p+d block-diag] via 2
     gather DMAs from the DRAM bounce. Structural zeros (kblk/vblk alternating
     const tiles) memset once per execution on GPSIMD.
  4. Scores^T: 64 pair matmuls (k=128, m=32 col-quadrant-rotated, n=16,
     stationary KBLK slab, moving QSTK slice) -> sco psum [(u,p,j), (gg,i)].
  5. Softmax: ACT exp -> E bf16; Z = mask-matmul (sums j over partitions,
     padded to m=32); DVE reciprocal; Zb = maskT-matmul broadcast; A = E*Zb
     (DVE); A2 restage to partition base 0 (4 DVE copies).
  6. AV: 64 pair matmuls (k=32, m=128, n=16, all at PE quadrant (0,0):
     stationary VBLK slab, moving A2 slice) -> avp psum [(p,d), 16g+i].
  7. One strided ACT copy avp -> out2T [128 (a,d), (c', sub, G, i)].
  8. Final projection: stationary WoPT chunks, moving out2T -> psum
     [128 f, 512 s'], + bo (DVE) -> out_d (1024, 2048) f32.
Host: transposes x/weights, permutes Wo rows, post-scatters out columns
(col = 16G + i -> row i*256 + s0/16 + G).
Note: row-quadrant tile_position (32u, 0) on AV matmuls compiles but faults
real HW (NRT_EXEC_UNIT_UNRECOVERABLE) — hence the A2 restage to base 0.
"""

import math
import numpy as np
import ml_dtypes

ROWS = 2048
NB, SB, EB, HB, DB = 4, 4096, 1024, 16, 64

_CACHE = {}


def _split_waits_json(bir_bytes):
    """This env's walrus accepts only ONE embedded sync-wait per TPB
    instruction (NEURON_ISA_TPB_EVENTS has a single wait slot) but Tile emits
    several. Split excess on_wait entries onto standalone EventSemaphore
    instructions inserted just before, on the same engine — semantically
    identical on in-order engine queues."""
    import json
    d = json.loads(bir_bytes)
    for fn in d.get('functions', []):
        for bb in (fn.get('basic_blocks') or fn.get('blocks') or []):
            out = []
            for inst in bb.get('instructions', []):
                si = inst.get('sync_info')
                w = (si or {}).get('on_wait') or []
                if len(w) > 1:
                    for k, extra in enumerate(w[:-1]):
                        out.append({
                            'debug': inst.get('debug', 0),
                            'engine': inst['engine'],
                            'ins': [], 'outs': [],
                            'name': f"{inst['name']}-sw{k}",
                            'opcode': 'EventSemaphore',
                            'sync_info': {'on_wait': [extra], 'on_update': []},
                        })
                    si['on_wait'] = [w[-1]]
                out.append(inst)
            bb['instructions'] = out
    return json.dumps(d).encode()


def _install_birpatch():
    import concourse.bass_utils as bu
    import concourse.bass2jax as b2j
    if getattr(bu.compile_bir_kernel, '_waitsplit', False):
        return
    orig = bu.compile_bir_kernel

    def patched(bir_json, tmpdir, neff_name="file.neff"):
        return orig(_split_waits_json(bir_json), tmpdir, neff_name)

    patched._waitsplit = True
    bu.compile_bir_kernel = patched
    b2j.compile_bir_kernel = patched


def _build_bass(reps=1):
    import os
    import concourse.bass as bass
    import concourse.tile as tile
    from concourse import mybir
    _ab = set(os.environ.get("KABLATE", "").split(","))

    bf16 = mybir.dt.bfloat16
    f32 = mybir.dt.float32
    AF = mybir.ActivationFunctionType

    nc = bass.Bass(trn_type="TRN2")
    xt_d = nc.declare_dram_parameter("xt", [1024, ROWS], bf16, isOutput=False)
    wqT_d = nc.declare_dram_parameter("wqt", [1024, 1024], bf16, isOutput=False)
    wkT_d = nc.declare_dram_parameter("wkt", [1024, 1024], bf16, isOutput=False)
    wvT_d = nc.declare_dram_parameter("wvt", [1024, 1024], bf16, isOutput=False)
    woT_d = nc.declare_dram_parameter("wot", [1024, 1024], bf16, isOutput=False)
    bias_d = nc.declare_dram_parameter("bias", [128, 1048], f32, isOutput=False)
    mask_d = nc.declare_dram_parameter("mask", [128, 32], bf16, isOutput=False)
    maskT_d = nc.declare_dram_parameter("maskt", [32, 128], bf16, isOutput=False)
    out_d = nc.declare_dram_parameter("out", [1024, ROWS], f32, isOutput=True)

    from contextlib import ExitStack
    with ExitStack() as ctx:
        tc = ctx.enter_context(tile.TileContext(nc))
        const = ctx.enter_context(tc.tile_pool(name="const", bufs=1))
        work = ctx.enter_context(tc.tile_pool(name="work", bufs=2))
        vnp = ctx.enter_context(tc.tile_pool(name="vnp", bufs=2))
        ppj = ctx.enter_context(tc.tile_pool(name="ppj", bufs=2, space="PSUM"))
        pvj = ctx.enter_context(tc.tile_pool(name="pvj", bufs=1, space="PSUM"))
        psc = ctx.enter_context(tc.tile_pool(name="psc", bufs=2, space="PSUM"))
        ps1 = ctx.enter_context(tc.tile_pool(name="ps1", bufs=1, space="PSUM"))
        drp = ctx.enter_context(tc.tile_pool(name="drp", bufs=4, space="DRAM"))

        if reps != 1:
            ctx.enter_context(tc.For_i(0, reps))

        # ---- persistent tensors (reloaded every rep for honest timing) ----
        wq_sb = const.tile([128, 8192], bf16, tag="wq")
        wk_sb = const.tile([128, 8192], bf16, tag="wk")
        wv_sb = const.tile([128, 8192], bf16, tag="wv")
        wo_sb = const.tile([128, 8192], bf16, tag="wo")
        bias_sb = const.tile([128, 1048], f32, tag="bias")
        mask_sb = const.tile([128, 32], bf16, tag="mask")
        maskT_sb = const.tile([32, 128], bf16, tag="maskt")
        out2t = const.tile([128, 16384], bf16, tag="o2t", name="o2t")
        kblks = [const.tile([128, 2048], bf16, tag=f"kblk{b}", name=f"kblk{b}")
                 for b in range(2)]
        vblks = [const.tile([32, 8192], bf16, tag=f"vblk{b}", name=f"vblk{b}")
                 for b in range(2)]
        for b in range(2):
            nc.gpsimd.memset(kblks[b][:], 0.0)
            nc.gpsimd.memset(vblks[b][:], 0.0)

        # weight layout: sb[p, 1024*ce + 128*cf + m] = W_T[128*ce + p, 128*cf + m]
        for sb, d in ((wq_sb, wqT_d), (wk_sb, wkT_d), (wv_sb, wvT_d), (wo_sb, woT_d)):
            nc.sync.dma_start(
                sb[:].rearrange("p (ce f) -> p ce f", ce=8),
                d[:].rearrange("(ce p) f -> p ce f", ce=8),
            )
        nc.sync.dma_start(bias_sb[:], bias_d[:])
        nc.sync.dma_start(mask_sb[:], mask_d[:])
        nc.sync.dma_start(maskT_sb[:], maskT_d[:])
        bqT = bias_sb[:, 0:8]
        bkT = bias_sb[:, 8:16]
        boT = bias_sb[:, 16:24]
        bvR = bias_sb[:, 24:1048]

        def _emit_final(fg):
            for cf in range(8):
                pf = ps1.tile([128, 512], f32, tag="zpf", name="pf")
                for cp in range(8):
                    nc.tensor.matmul(
                        pf[:], wo_sb[:, 1024 * cp + 128 * cf:1024 * cp + 128 * (cf + 1)],
                        out2t[:, 2048 * cp + 512 * fg:2048 * cp + 512 * (fg + 1)],
                        start=(cp == 0), stop=(cp == 7))
                osb = work.tile([128, 512], f32, tag="osb")
                nc.vector.tensor_scalar_add(osb[:], pf[:], boT[:, cf:cf + 1])
                nc.sync.dma_start(out_d[128 * cf:128 * (cf + 1), 512 * fg:512 * (fg + 1)], osb[:])

        for sg in range(4):
            # ---- load xT group: [128, (ce, s 512)] ----
            xt_sb = work.tile([128, 4096], bf16, tag="xt")
            nc.sync.dma_start(
                xt_sb[:].rearrange("p (ce s) -> p ce s", ce=8),
                xt_d[:].rearrange("(ce p) s -> p ce s", ce=8)[:, :, 512 * sg:512 * (sg + 1)],
            )
            # ---- QT / KT projections (transposed: stationary = weights) ----
            qt_sg = work.tile([128, 4096], bf16, tag="qt")
            kt_sg = work.tile([128, 4096], bf16, tag="kt")
            for w_sb, bT, dst in ((wq_sb, bqT, qt_sg), (wk_sb, bkT, kt_sg)):
                for cf in range(8):
                    pp = ppj.tile([128, 512], f32, tag="pp", name="pp")
                    for ce in range(8):
                        nc.tensor.matmul(
                            pp[:], w_sb[:, 1024 * ce + 128 * cf:1024 * ce + 128 * (cf + 1)],
                            xt_sb[:, 512 * ce:512 * (ce + 1)],
                            start=(ce == 0), stop=(ce == 7))
                    nc.vector.tensor_scalar_add(
                        dst[:, 512 * cf:512 * (cf + 1)], pp[:], bT[:, cf:cf + 1])
            # ---- V projection (natural: stationary = xT chunk) ----
            vns = []
            for sl in range(4):
                vn = vnp.tile([128, 1024], bf16, tag="vn")
                for h in range(2):
                    pv = pvj.tile([128, 512], f32, tag="pv", name="pv")
                    for ce in range(8):
                        nc.tensor.matmul(
                            pv[:],
                            xt_sb[:, 512 * ce + 128 * sl:512 * ce + 128 * (sl + 1)],
                            wv_sb[:, 1024 * ce + 512 * h:1024 * ce + 512 * (h + 1)],
                            start=(ce == 0), stop=(ce == 7))
                    nc.vector.tensor_add(vn[:, 512 * h:512 * (h + 1)], pv[:],
                                         bvR[:, 512 * h:512 * (h + 1)])
                vdr = drp.tile([128, 1024], bf16, tag="vdr")
                nc.sync.dma_start(vdr[:], vn[:])
                vns.append(vdr)

            for sl in range(4):
                sub = 4 * sg + sl
                vdr = vns[sl]
                # ---- VBLK [32 (p,j), 128g+64p+d] via 2 gather DMAs (DRAM src) ----
                vblk = vblks[sub % 2]
                for p in range(2 if "novblk" not in _ab else 0):
                    nc.sync.dma_start(
                        vblk[:][16 * p:16 * (p + 1), :].rearrange(
                            "P (g two d) -> P g two d", g=64, two=2)[:, :, p, :],
                        vdr[:].rearrange("(G cp t) (j d) -> t j G cp d",
                                         G=8, cp=8, t=2, j=16)[p],
                    )
                # ---- QSTK / KBLK via 4 strided DVE copies each ----
                qstk = work.tile([128, 1024], bf16, tag="qstk", name="qstk")
                kblk = kblks[sub % 2]
                for p in range(2):
                    for a in range(2):
                        src_q = qt_sg[:][64 * a:64 * (a + 1), :].rearrange(
                            "P (c sl G cp t) -> P sl t G cp c",
                            c=8, sl=4, G=8, cp=8, t=2)[:, sl, p]
                        dst_q = qstk[:][64 * p:64 * (p + 1), :].rearrange(
                            "P (G cp c t) -> P t G cp c", G=8, cp=8, c=8, t=2)[:, a]
                        nc.vector.tensor_copy(dst_q, src_q)
                        src_k = kt_sg[:][64 * a:64 * (a + 1), :].rearrange(
                            "P (c sl G cp t) -> P sl t G cp c",
                            c=8, sl=4, G=8, cp=8, t=2)[:, sl, p]
                        dst_k = kblk[:][64 * p:64 * (p + 1), :].rearrange(
                            "P (G cp two c t) -> P two t G cp c",
                            G=8, cp=8, two=2, c=8, t=2)[:, p, a]
                        nc.vector.tensor_copy(dst_k, src_k)
                # ---- scores^T: 64 pair matmuls ----
                sco = psc.tile([128, 256], f32, tag="sco", name="sco")
                if "nosco" in _ab:
                    nc.scalar.activation(sco[:], qstk[:, 0:256], func=AF.Copy)
                else:
                    for g in range(64):
                        u, gg = g % 4, g // 4
                        nc.tensor.matmul(
                            sco[32 * u:32 * (u + 1), 16 * gg:16 * (gg + 1)],
                            kblk[:, 32 * g:32 * (g + 1)],
                            qstk[:, 16 * g:16 * (g + 1)],
                            start=True, stop=True, tile_position=(0, 32 * u))
                # ---- softmax ----
                e_sb = work.tile([128, 256], bf16, tag="esb")
                nc.scalar.activation(e_sb[:], sco[:], func=AF.Exp)
                a_sb = work.tile([128, 256], bf16, tag="asb")
                if "noz" in _ab:
                    nc.vector.tensor_copy(a_sb[:], e_sb[:])
                else:
                    zp = ps1.tile([128, 512], f32, tag="zpf", name="zp")
                    nc.tensor.matmul(zp[0:32, 0:256], mask_sb[:], e_sb[:],
                                     start=True, stop=True)
                    zr = work.tile([32, 256], bf16, tag="zr")
                    nc.vector.memset(zr[:], 0.0)
                    with nc.allow_low_precision(reason="1/Z in bf16: 0.4% on softmax scale"):
                        nc.vector.reciprocal(zr[0:8, :], zp[0:8, 0:256])
                    nc.tensor.matmul(zp[:, 256:512], maskT_sb[:], zr[:],
                                     start=True, stop=True)
                    nc.vector.tensor_mul(a_sb[:], e_sb[:], zp[:, 256:512])
                # ---- A2: bands stacked at partition base 0 ----
                a2 = work.tile([32, 1024], bf16, tag="a2")
                for u in range(4):
                    nc.vector.tensor_copy(a2[:, 256 * u:256 * (u + 1)],
                                          a_sb[32 * u:32 * (u + 1), :])
                # ---- AV: 64 pair matmuls ----
                avp = ps1.tile([128, 1024], f32, tag="avp", name="avp")
                if "noav" in _ab:
                    for q in range(4):
                        nc.scalar.activation(avp[:, 256 * q:256 * (q + 1)], a_sb[:],
                                             func=AF.Copy)
                else:
                    for g in range(64):
                        u, gg = g % 4, g // 4
                        nc.tensor.matmul(
                            avp[:, 16 * g:16 * (g + 1)],
                            vblk[:, 128 * g:128 * (g + 1)],
                            a2[:, 256 * u + 16 * gg:256 * u + 16 * (gg + 1)],
                            start=True, stop=True)
                # ---- evac to out2T (one strided ACT op) ----
                nc.scalar.activation(
                    out2t[:].rearrange("P (cp sb G i) -> P sb cp G i",
                                       cp=8, sb=16, G=8, i=16)[:, sub],
                    avp[:].rearrange("P (G cp i) -> P cp G i", G=8, cp=8),
                    func=AF.Copy)

            # ---- final projection for this group's s' columns ----
            _emit_final(sg)
    return nc


def _host_prep(x, Wq, bq, Wk, bk, Wv, bv, Wo, bo):
    """Returns per-core input maps."""
    xf = np.ascontiguousarray(x.reshape(NB * SB, EB))
    WqT = np.ascontiguousarray((Wq / 8.0).T).astype(ml_dtypes.bfloat16)
    WkT = np.ascontiguousarray(Wk.T).astype(ml_dtypes.bfloat16)
    WvT = np.ascontiguousarray(Wv.T).astype(ml_dtypes.bfloat16)
    WoPT = np.zeros((1024, 1024), np.float32)
    for cp in range(8):
        for a in range(2):
            w = 2 * cp + a
            WoPT[128 * cp + 64 * a:128 * cp + 64 * a + 64, :] = Wo[:, 64 * w:64 * (w + 1)].T
    WoPT = WoPT.astype(ml_dtypes.bfloat16)
    bias = np.zeros((128, 1048), np.float32)
    bias[:, 0:8] = (bq / 8.0).reshape(8, 128).T
    bias[:, 8:16] = bk.reshape(8, 128).T
    bias[:, 16:24] = bo.reshape(8, 128).T
    bias[:, 24:1048] = np.tile(bv[None, :], (128, 1))
    MASK = np.zeros((128, 32), np.float32)
    for u in range(4):
        for p in range(2):
            MASK[32 * u + 16 * p:32 * u + 16 * (p + 1), 2 * u + p] = 1.0
    MASKb = MASK.astype(ml_dtypes.bfloat16)
    MASKTb = np.ascontiguousarray(MASK.T).astype(ml_dtypes.bfloat16)
    in_maps = []
    for core in range(8):
        n, s0 = core // 2, (core % 2) * ROWS
        xs = xf[n * SB + s0:n * SB + s0 + ROWS]
        xT = np.ascontiguousarray(xs.T).astype(ml_dtypes.bfloat16)
        in_maps.append({"xt": xT, "wqt": WqT, "wkt": WkT, "wvt": WvT,
                        "wot": WoPT, "bias": bias, "mask": MASKb, "maskt": MASKTb})
    return in_maps


def _gather_out(core_outs):
    """core_outs: list of 8 per-core 'out' arrays (1024, 2048) -> full (N,S,E).
    out col = 16*G + i -> row i*256 + s0/16 + G."""
    out = np.zeros((NB, SB, EB), np.float32)
    cols = np.arange(ROWS)
    G, i = cols // 16, cols % 16
    for core in range(8):
        n, s0 = core // 2, (core % 2) * ROWS
        fT = np.asarray(core_outs[core])  # (1024, 2048)
        rows = i * 256 + (s0 // 16 + G)
        out[n, rows, :] = fT.T
    return out


def kernel(x, Wq, bq, Wk, bk, Wv, bv, Wo, bo):
    _install_birpatch()
    from concourse.bass_utils import run_bass_kernel_spmd

    if "nc" not in _CACHE:
        _CACHE["nc"] = _build_bass()
    nc = _CACHE["nc"]
    in_maps = _host_prep(np.asarray(x, np.float32), *[np.asarray(a, np.float32)
                         for a in (Wq, bq, Wk, bk, Wv, bv, Wo, bo)])
    res = run_bass_kernel_spmd(nc, in_maps, list(range(8)))
    return _gather_out([res.results[core]["out"] for core in range(8)])
